# revision 1
# baseline (speedup 1.0000x reference)
"""Trainium2 Bass kernel for nn_AttentionNet (pooling / ridge regime).

Model (per batch b of B=128, L=512, D=300, H=200, V=50000):
  word_emb = emb_table[words]                          [B,L,D]
  subj_emb = max over l with subj_pos[b,l]==0 of word_emb (else -1e12)
  obj_emb  = same with obj_pos
  hid  = tanh(word_emb @ w1[:D] + subj_emb @ w1[D:] + b1)
  attn = softmax(hid @ w2, axis=l)    (b2 dropped: softmax shift-invariant)
  subj_attn = sum_l attn * word_emb   (obj_attn identical -- source bug)
  out = relu(relu(cat([subj_attn, subj_attn, subj_emb, obj_emb]) @ mw1 + mb1) @ mw2 + mb2)

Sharding: pure data parallel, 16 batches per core on 8 cores; embedding
table and the small weights replicated.

Device plan per core (16 batches = 16 token-tiles of 512):
  - bulk gather via gpsimd.dma_gather (int16 indices).  The vocabulary
    exceeds int16 range, so each batch's tokens are sorted by word id
    (attention + pools are order-invariant within a batch) and split into
    the 256 smallest / 256 largest; the low halves of all batches are
    gathered from table[0:32768] and the high halves from
    table[V-32768:V], giving pure int16 indices with zero waste.
  - masked max-pools computed from the gathered embeddings: per-token
    additive masks (-2e12 for suppressed positions) via ACT/GPSIMD, a
    pairwise max tree on DVE/GPSIMD, PE transposes, a segmented
    reduce_max, then a final clamp to -1e12 which restores bit-exact
    semantics even for all-masked rows.
  - attention scores via fp32r matmuls on D-major PE-transposed
    embeddings, softmax on-chip, attention-weighted sum with the
    attention column as the stationary matmul operand.
  - 2-layer output MLP with the duplicated subj_attn block pre-folded
    into mw1 on the host (rows 0:300 += rows 300:600).
"""

import numpy as np

import concourse.bass as bass
import concourse.bacc as bacc
import concourse.mybir as mybir
import concourse.tile as tile
from concourse.masks import make_identity
from contextlib import ExitStack

F32 = mybir.dt.float32
F32R = mybir.dt.float32r
I16 = mybir.dt.int16

NEG_INF = 1e12      # reference constant
MASK_ADD = -2e12    # additive mask; clamped back to -NEG_INF at the end

# ---------------------------------------------------------------- config


class Cfg:
    def __init__(self, B=128, L=512, D=300, H=200, V=50000, NCORES=8,
                 PT=128, CW=128, HCW=100, use_f32r=True, gather_split=4):
        self.B, self.L, self.D, self.H, self.V = B, L, D, H, V
        self.NCORES = NCORES
        self.use_f32r = use_f32r
        self.BC = B // NCORES          # batches per core
        self.PT = PT                   # token subtile (partitions)
        self.NSUB = L // PT            # subtiles per batch (must be even)
        self.NS = self.BC * self.NSUB  # token subtiles per core
        self.T = self.BC * L           # tokens per core
        self.CW = CW                   # D-chunk width
        self.HCW = HCW                 # H-chunk width
        self.gather_split = gather_split
        assert L % PT == 0 and H % HCW == 0 and self.NSUB % 2 == 0
        # gather element size: row bytes padded to 256B multiples
        self.E = -(-D * 4 // 256) * 64
        # int16-addressable split of the vocabulary
        self.LO_MAX = min(V, 32768)    # low table = rows [0, LO_MAX)
        self.HB = max(V - 32768, 0)    # high table = rows [HB, V)
        # exact chunks of D (last may be narrow)
        self.dch = []
        s = 0
        while s < D:
            self.dch.append((s, min(CW, D - s)))
            s += CW
        self.hch = [(i * HCW, HCW) for i in range(H // HCW)]
        self.nd = len(self.dch)
        self.nh = len(self.hch)
        # transpose window start per chunk (narrow last chunk reads an
        # overlapping window ending at the padded width E; its rows sit at
        # a 32-aligned base so downstream APs stay legal)
        self.ov0 = [min(i * CW, self.E - CW) for i in range(self.nd)]
        self.r0 = [self.dch[i][0] - self.ov0[i] for i in range(self.nd)]
        for r, (d0, dn) in zip(self.r0, self.dch):
            assert r in (0, 32, 64, 96) and (r == 0 or dn <= max(32, 128 - r)), (r, dn)
        # pool-transpose source width padded to nd*CW (extra cols memset)
        self.DP = self.nd * CW

    def subtiles(self, b):
        """Global subtile ids of batch b: low half then high half."""
        h = self.NSUB // 2
        lo = [h * b + k for k in range(h)]
        hi = [self.NS // 2 + h * b + k for k in range(h)]
        return lo + hi


# ------------------------------------------------------------- device IR


def build_nc(cfg: Cfg):
    c = cfg
    FR = F32R if c.use_f32r else F32
    nc = bacc.Bacc(num_swdge_queues=4)

    NH16 = (c.T // 2) // 16
    idxlo_d = nc.declare_dram_parameter("idx_lo", [128, NH16], I16, isOutput=False)
    idxhi_d = nc.declare_dram_parameter("idx_hi", [128, NH16], I16, isOutput=False)
    table = nc.declare_dram_parameter("table", [c.V, c.E], FR, isOutput=False)
    madd_d = nc.declare_dram_parameter("madd", [c.PT, 2, c.BC, c.NSUB], F32, isOutput=False)
    w1a_d = nc.declare_dram_parameter("w1a", [c.D, c.H], F32, isOutput=False)
    w1b_d = nc.declare_dram_parameter("w1b", [c.D, c.H], F32, isOutput=False)
    b1_d = nc.declare_dram_parameter("b1", [c.H, 1], F32, isOutput=False)
    w2_d = nc.declare_dram_parameter("w2", [c.H, 1], F32, isOutput=False)
    mw1_d = nc.declare_dram_parameter("mw1e", [3 * c.D, c.H], F32, isOutput=False)
    mb1_d = nc.declare_dram_parameter("mb1", [c.H, 1], F32, isOutput=False)
    mw2_d = nc.declare_dram_parameter("mw2", [c.H, c.H], F32, isOutput=False)
    mb2_d = nc.declare_dram_parameter("mb2", [c.H, 1], F32, isOutput=False)
    out_d = nc.declare_dram_parameter("out", [c.nh, c.HCW, c.BC], F32, isOutput=True)

    with tile.TileContext(nc) as tc, ExitStack() as ctx:
        sb = ctx.enter_context(tc.tile_pool(name="sb", bufs=1))

        # ---- persistent SBUF tiles
        ixl_sb = sb.tile([128, NH16], I16)
        ixh_sb = sb.tile([128, NH16], I16)
        emb_tok = sb.tile([c.PT, c.NS, c.E], FR)
        madd_sb = sb.tile([c.PT, 2, c.BC, c.NSUB], F32)
        w1a_sb = sb.tile([c.CW, c.nd, c.H], F32)
        w1a_r = sb.tile([c.CW, c.nd, c.H], FR)
        w1b_sb = sb.tile([c.CW, c.nd, c.H], F32)
        w2_sb = sb.tile([c.HCW, c.nh], F32)
        w2_r = sb.tile([c.HCW, c.nh], FR)
        b1_sb = sb.tile([c.HCW, c.nh], F32)
        mw1_sb = sb.tile([c.CW, 3 * c.nd, c.H], F32)
        mb1_sb = sb.tile([c.HCW, c.nh], F32)
        mw2_sb = sb.tile([c.HCW, c.nh, c.H], F32)
        mb2_sb = sb.tile([c.HCW, c.nh], F32)
        ident = sb.tile([c.PT, c.PT], F32)
        identr = sb.tile([c.PT, c.PT], FR)
        pooled = sb.tile([c.CW, 2, c.BC, c.nd], F32)   # [dlow, mask, b, chunk]
        bias_sb = sb.tile([c.HCW, c.nh, c.BC], F32)
        scores = sb.tile([c.BC, c.L], F32)
        attn = sb.tile([c.BC, c.L], F32)
        attn_t = sb.tile([c.PT, c.NSUB, c.BC], FR)
        sattn = sb.tile([c.BC, c.D], F32)
        satd = sb.tile([c.CW, c.nd, c.BC], F32)
        smax = sb.tile([c.BC, 1], F32)
        nsmax = sb.tile([c.BC, 1], F32)
        ssum = sb.tile([c.BC, 1], F32)
        srec = sb.tile([c.BC, 1], F32)
        o1_sb = sb.tile([c.HCW, c.nh, c.BC], F32)
        out_sb = sb.tile([c.HCW, c.nh, c.BC], F32)

        # ---- load indices & weights
        nc.sync.dma_start(out=ixl_sb[:], in_=idxlo_d[:])
        nc.sync.dma_start(out=ixh_sb[:], in_=idxhi_d[:])
        nc.sync.dma_start(out=madd_sb[:], in_=madd_d[:])
        for ci, (d0, dn) in enumerate(c.dch):
            r0 = c.r0[ci]
            nc.sync.dma_start(out=w1a_sb[r0:r0 + dn, ci, :], in_=w1a_d[d0:d0 + dn, :])
            nc.vector.tensor_copy(out=w1a_r[r0:r0 + dn, ci, :],
                                  in_=w1a_sb[r0:r0 + dn, ci, :])
            nc.sync.dma_start(out=w1b_sb[0:dn, ci, :], in_=w1b_d[d0:d0 + dn, :])
            for blk in range(3):
                nc.sync.dma_start(out=mw1_sb[0:dn, blk * c.nd + ci, :],
                                  in_=mw1_d[blk * c.D + d0:blk * c.D + d0 + dn, :])
        for hi, (h0, hn) in enumerate(c.hch):
            nc.sync.dma_start(out=w2_sb[0:hn, hi:hi + 1], in_=w2_d[h0:h0 + hn, :])
            nc.vector.tensor_copy(out=w2_r[0:hn, hi:hi + 1], in_=w2_sb[0:hn, hi:hi + 1])
            nc.sync.dma_start(out=b1_sb[0:hn, hi:hi + 1], in_=b1_d[h0:h0 + hn, :])
            nc.sync.dma_start(out=mb1_sb[0:hn, hi:hi + 1], in_=mb1_d[h0:h0 + hn, :])
            nc.sync.dma_start(out=mb2_sb[0:hn, hi:hi + 1], in_=mb2_d[h0:h0 + hn, :])
            nc.sync.dma_start(out=mw2_sb[0:hn, hi, :], in_=mw2_d[h0:h0 + hn, :])
        make_identity(nc, ident[:])
        nc.vector.tensor_copy(out=identr[:], in_=ident[:])

        # ---- bulk gathers: low halves -> subtiles [0, NS/2), high halves after
        NHALF = c.T // 2
        nsp = c.gather_split
        npc = NHALF // nsp
        assert npc % 128 == 0, (NHALF, nsp)
        for k in range(nsp):
            i0, s0 = k * (npc // 16), k * (npc // 128)
            nc.gpsimd.dma_gather(
                out_ap=emb_tok[:, s0:s0 + npc // 128, :], in_ap=table[0:c.LO_MAX, :],
                idxs_ap=ixl_sb[:, i0:i0 + npc // 16], num_idxs=npc, num_idxs_reg=npc,
                elem_size=c.E, single_packet=False, queue_num=(2 * k) % 4 if c.use_f32r else 0)
            nc.gpsimd.dma_gather(
                out_ap=emb_tok[:, c.NS // 2 + s0:c.NS // 2 + s0 + npc // 128, :],
                in_ap=table[c.HB:c.V, :],
                idxs_ap=ixh_sb[:, i0:i0 + npc // 16], num_idxs=npc, num_idxs_reg=npc,
                elem_size=c.E, single_packet=False, queue_num=(2 * k + 1) % 4)

        # ---- grouped main loop: pools -> group bias -> dense hid/scores.
        # Groups of GB batches keep the PE stream dense (HAM stays warm) and
        # let the ACT/DVE pool work of group g+1 overlap the PE work of g.
        GB = min(4, c.BC)
        NG = c.BC // GB

        def pool_rhs(m, ci, bsl=slice(None)):
            dn = c.dch[ci][1]
            return pooled[0:dn, m, bsl, ci]

        with tc.tile_pool(name="mkpool", bufs=3) as mkpool, \
             tc.tile_pool(name="mxpool", bufs=3) as mxpool, \
             tc.tile_pool(name="ppool", bufs=1, space="PSUM") as ppool, \
             tc.tile_pool(name="bpool", bufs=1, space="PSUM") as bpool, \
             tc.tile_pool(name="tpool", bufs=1, space="PSUM") as tpool, \
             tc.tile_pool(name="hpool", bufs=2, space="PSUM") as hpool, \
             tc.tile_pool(name="spool", bufs=1, space="PSUM") as spool, \
             tc.tile_pool(name="epool", bufs=2) as epool, \
             tc.tile_pool(name="hspool", bufs=2) as hspool, \
             tc.tile_pool(name="srpool", bufs=3) as srpool:
            for g in range(NG):
                gsl = slice(g * GB, (g + 1) * GB)
                # -- pools for the group
                for b in range(g * GB, (g + 1) * GB):
                    subs = c.subtiles(b)
                    for m in range(2):
                        masked = mkpool.tile([c.PT, c.NSUB, c.D], F32, tag="masked")
                        for si, s in enumerate(subs):
                            if m == 0:
                                nc.scalar.activation(
                                    out=masked[:, si, :],
                                    in_=emb_tok[:, s, 0:c.D].bitcast(F32),
                                    func=mybir.ActivationFunctionType.Identity,
                                    bias=madd_sb[:, m, b, si:si + 1], scale=1.0)
                            else:
                                nc.vector.tensor_scalar(
                                    out=masked[:, si, :],
                                    in0=emb_tok[:, s, 0:c.D].bitcast(F32),
                                    scalar1=madd_sb[:, m, b, si:si + 1],
                                    scalar2=None, op0=mybir.AluOpType.add)
                        h = c.NSUB // 2
                        maxed = mxpool.tile([c.PT, c.DP], F32, tag="maxed")
                        t1 = mkpool.tile([c.PT, h, c.D], F32, tag="t1")
                        nc.vector.tensor_tensor(out=t1[:], in0=masked[:, 0:h, :],
                                                in1=masked[:, h:c.NSUB, :],
                                                op=mybir.AluOpType.max)
                        for q in range(h.bit_length() - 1):
                            hh = h >> (q + 1)
                            nc.vector.tensor_tensor(
                                out=t1[:, 0:hh, :], in0=t1[:, 0:hh, :],
                                in1=t1[:, hh:2 * hh, :], op=mybir.AluOpType.max)
                        nc.gpsimd.memset(maxed[:, c.D:c.DP], MASK_ADD)
                        nc.vector.tensor_copy(out=maxed[:, 0:c.D], in_=t1[:, 0, :])
                        pp = ppool.tile([c.CW, c.nd, c.PT], F32, tag="pp")
                        for ci in range(c.nd):
                            nc.tensor.transpose(
                                out=pp[:, ci, :],
                                in_=maxed[:, ci * c.CW:(ci + 1) * c.CW],
                                identity=ident[:])
                        nc.vector.tensor_reduce(
                            out=pooled[:, m, b, :], in_=pp[:],
                            axis=mybir.AxisListType.X, op=mybir.AluOpType.max)
                # -- clamp restores exact -1e12 for all-masked rows
                nc.vector.tensor_scalar_max(
                    out=pooled[:, :, gsl, :], in0=pooled[:, :, gsl, :],
                    scalar1=-NEG_INF)
                # -- tanh bias for the group: w1b^T subj_emb + b1
                for hi, (h0, hn) in enumerate(c.hch):
                    pb = bpool.tile([c.HCW, GB], F32, tag="pb")
                    for ci, (d0, dn) in enumerate(c.dch):
                        nc.tensor.matmul(
                            out=pb[0:hn, :],
                            lhsT=w1b_sb[0:dn, ci, h0:h0 + hn],
                            rhs=pool_rhs(0, ci, gsl),
                            start=(ci == 0), stop=(ci == c.nd - 1))
                    nc.scalar.activation(
                        out=bias_sb[0:hn, hi, gsl], in_=pb[0:hn, :],
                        func=mybir.ActivationFunctionType.Identity,
                        bias=b1_sb[0:hn, hi:hi + 1], scale=1.0)
                # -- D-major transposes + copies for the group
                embds = []
                for b in range(g * GB, (g + 1) * GB):
                    subs = c.subtiles(b)
                    pt = tpool.tile([c.CW, c.nd, c.L], FR, tag="pt")
                    for ci in range(c.nd):
                        o0 = c.ov0[ci]
                        for si, s in enumerate(subs):
                            nc.tensor.transpose(
                                out=pt[:, ci, si * c.PT:(si + 1) * c.PT],
                                in_=emb_tok[:, s, o0:o0 + c.CW],
                                identity=identr[:])
                    emb_d = epool.tile([c.CW, c.nd, c.L], FR, tag="embd")
                    nc.scalar.copy(out=emb_d[:, 0, :], in_=pt[:, 0, :])
                    nc.vector.tensor_copy(out=emb_d[:, 1:c.nd, :], in_=pt[:, 1:c.nd, :])
                    embds.append(emb_d)
                # -- dense hid + scores matmul stream for the group
                for bi, b in enumerate(range(g * GB, (g + 1) * GB)):
                    emb_d = embds[bi]
                    hid = hspool.tile([c.HCW, c.nh, c.L], FR, tag="hid")
                    for hi, (h0, hn) in enumerate(c.hch):
                        ph = hpool.tile([c.HCW, c.L], F32, tag="ph")
                        for ci, (d0, dn) in enumerate(c.dch):
                            r0 = c.r0[ci]
                            nc.tensor.matmul(
                                out=ph[0:hn, :],
                                lhsT=w1a_r[r0:r0 + dn, ci, h0:h0 + hn],
                                rhs=emb_d[r0:r0 + dn, ci, :],
                                start=(ci == 0), stop=(ci == c.nd - 1))
                        nc.scalar.activation(
                            out=hid[0:hn, hi, :], in_=ph[0:hn, :],
                            func=mybir.ActivationFunctionType.Tanh,
                            bias=bias_sb[0:hn, hi, b:b + 1], scale=1.0)
                    ps = spool.tile([1, c.L], F32, tag="ps")
                    for hi, (h0, hn) in enumerate(c.hch):
                        nc.tensor.matmul(
                            out=ps[:], lhsT=w2_r[0:hn, hi:hi + 1],
                            rhs=hid[0:hn, hi, :],
                            start=(hi == 0), stop=(hi == c.nh - 1))
                    srow = srpool.tile([1, c.L], F32, tag="srow")
                    nc.vector.tensor_copy(out=srow[:], in_=ps[:])
                    nc.sync.dma_start(out=scores[b:b + 1, :], in_=srow[:])

        # ---- softmax over L for all batches
        nc.vector.tensor_reduce(out=smax[:], in_=scores[:],
                                axis=mybir.AxisListType.X, op=mybir.AluOpType.max)
        nc.vector.tensor_scalar_mul(out=nsmax[:], in0=smax[:], scalar1=-1.0)
        nc.scalar.activation(out=attn[:], in_=scores[:],
                             func=mybir.ActivationFunctionType.Exp,
                             bias=nsmax[:, 0:1], scale=1.0)
        nc.vector.tensor_reduce(out=ssum[:], in_=attn[:],
                                axis=mybir.AxisListType.X, op=mybir.AluOpType.add)
        nc.vector.reciprocal(out=srec[:], in_=ssum[:])
        nc.vector.tensor_scalar_mul(out=attn[:], in0=attn[:], scalar1=srec[:, 0:1])

        # ---- transpose attn to token-major columns [PT, si, b]
        # column layout: attn[b, si*PT:...] -> attn_t[:, si, b]
        with tc.tile_pool(name="apool", bufs=2, space="PSUM") as apool:
            for si in range(c.NSUB):
                pa = apool.tile([c.PT, c.BC], F32, tag="pa")
                nc.tensor.transpose(out=pa[:],
                                    in_=attn[:, si * c.PT:(si + 1) * c.PT],
                                    identity=ident[0:c.BC, 0:c.BC])
                nc.vector.tensor_copy(out=attn_t[:, si, :], in_=pa[:])

        # ---- attention-weighted sum  -> sattn [b, D]
        with tc.tile_pool(name="wpool", bufs=4, space="PSUM") as wpool, \
             tc.tile_pool(name="wrpool", bufs=3) as wrpool:
            for b in range(c.BC):
                subs = c.subtiles(b)
                pw = wpool.tile([1, c.D], F32, tag="pw")
                for si, s in enumerate(subs):
                    nc.tensor.matmul(
                        out=pw[:],
                        lhsT=attn_t[:, si, b:b + 1],
                        rhs=emb_tok[:, s, 0:c.D],
                        start=(si == 0), stop=(si == c.NSUB - 1))
                wrow = wrpool.tile([1, c.D], F32, tag="wrow")
                nc.scalar.copy(out=wrow[:], in_=pw[:])
                nc.sync.dma_start(out=sattn[b:b + 1, :], in_=wrow[:])

        # ---- transpose sattn to D-major chunks [dlow, chunk, b]
        with tc.tile_pool(name="stpool", bufs=2, space="PSUM") as stpool:
            for ci, (d0, dn) in enumerate(c.dch):
                pst = stpool.tile([c.CW, c.BC], F32, tag="pst")
                nc.tensor.transpose(out=pst[0:dn, :], in_=sattn[:, d0:d0 + dn],
                                    identity=ident[0:c.BC, 0:c.BC])
                nc.vector.tensor_copy(out=satd[0:dn, ci, :], in_=pst[0:dn, :])

        # ---- output MLP (fp32; N=BC is small)
        with tc.tile_pool(name="mpool", bufs=2, space="PSUM") as mpool, \
             tc.tile_pool(name="m2pool", bufs=2, space="PSUM") as m2pool:
            nk = 3 * c.nd
            for hi, (h0, hn) in enumerate(c.hch):
                pm = mpool.tile([c.HCW, c.BC], F32, tag="pm")
                for blk in range(3):
                    for ci, (d0, dn) in enumerate(c.dch):
                        k = blk * c.nd + ci
                        if blk == 0:
                            rhs = satd[0:dn, ci, :]
                        else:
                            rhs = pool_rhs(blk - 1, ci)
                        nc.tensor.matmul(
                            out=pm[0:hn, :],
                            lhsT=mw1_sb[0:dn, k, h0:h0 + hn],
                            rhs=rhs, start=(k == 0), stop=(k == nk - 1))
                nc.scalar.activation(
                    out=o1_sb[0:hn, hi, :], in_=pm[0:hn, :],
                    func=mybir.ActivationFunctionType.Relu,
                    bias=mb1_sb[0:hn, hi:hi + 1], scale=1.0)
            for hi, (h0, hn) in enumerate(c.hch):
                pm2 = m2pool.tile([c.HCW, c.BC], F32, tag="pm2")
                for ki, (k0, kn) in enumerate(c.hch):
                    nc.tensor.matmul(
                        out=pm2[0:hn, :],
                        lhsT=mw2_sb[0:kn, ki, h0:h0 + hn],
                        rhs=o1_sb[0:kn, ki, :],
                        start=(ki == 0), stop=(ki == c.nh - 1))
                nc.scalar.activation(
                    out=out_sb[0:hn, hi, :], in_=pm2[0:hn, :],
                    func=mybir.ActivationFunctionType.Relu,
                    bias=mb2_sb[0:hn, hi:hi + 1], scale=1.0)
            for hi in range(c.nh):
                nc.sync.dma_start(out=out_d[hi], in_=out_sb[:, hi, :])

    nc.finalize()
    return nc


# ------------------------------------------------------------------ host


def wrap16(idx, n):
    """int16 index list -> [128, n/16] wrapped + replicated per Q7 core."""
    return np.ascontiguousarray(
        np.tile(idx.astype(np.int16).reshape(n // 16, 16).T, (8, 1)))


def host_prepare(cfg: Cfg, words, subj_pos, obj_pos, emb_table,
                 w1, b1, w2, b2, mw1, mb1, mw2, mb2):
    c = cfg
    words = np.asarray(words).astype(np.int64)
    subj_pos = np.asarray(subj_pos)
    obj_pos = np.asarray(obj_pos)
    f32 = lambda x: np.ascontiguousarray(np.asarray(x, dtype=np.float32))

    table = np.zeros((c.V, c.E), np.float32)
    table[:, :c.D] = np.asarray(emb_table, dtype=np.float32)

    w1 = f32(w1)
    w1a, w1b = w1[:c.D], w1[c.D:2 * c.D]
    mw1 = f32(mw1)
    mw1e = np.concatenate([mw1[0:c.D] + mw1[c.D:2 * c.D],
                           mw1[2 * c.D:3 * c.D], mw1[3 * c.D:4 * c.D]], axis=0)
    shared = {
        "table": table,
        "w1a": f32(w1a), "w1b": f32(w1b),
        "b1": f32(b1).reshape(c.H, 1),
        "w2": f32(w2).reshape(c.H, 1),
        "mw1e": f32(mw1e),
        "mb1": f32(mb1).reshape(c.H, 1),
        "mw2": f32(mw2),
        "mb2": f32(mb2).reshape(c.H, 1),
    }
    HALF = c.L // 2
    in_maps = []
    for core in range(c.NCORES):
        b0 = core * c.BC
        lo_list, hi_list = [], []
        madd = np.zeros((c.PT, 2, c.BC, c.NSUB), np.float32)
        for b in range(c.BC):
            w = words[b0 + b]
            order = np.argsort(w, kind="stable")
            ws = w[order]
            if ws[HALF - 1] >= c.LO_MAX or ws[HALF] < c.HB:
                raise RuntimeError(
                    f"batch {b0 + b}: vocab split infeasible "
                    f"(lo_max={ws[HALF - 1]}, hi_min={ws[HALF]})")
            lo_list.append(ws[:HALF])
            hi_list.append(ws[HALF:] - c.HB)
            # mask addends follow the same permutation; token rank r sits at
            # subtile si=r//PT (low half) / NSUB/2 + (r-HALF)//PT, partition r%PT
            for m, pos in ((0, subj_pos), (1, obj_pos)):
                pm = (np.asarray(pos[b0 + b])[order] != 0)
                av = np.where(pm, np.float32(MASK_ADD), np.float32(0.0))
                madd[:, m, b, :] = av.reshape(c.NSUB, c.PT).T
        idx_lo = np.concatenate(lo_list)
        idx_hi = np.concatenate(hi_list)
        in_maps.append({
            "idx_lo": wrap16(idx_lo, c.T // 2),
            "idx_hi": wrap16(idx_hi, c.T // 2),
            "madd": np.ascontiguousarray(madd),
            **shared})
    return in_maps


def assemble_output(cfg: Cfg, results):
    c = cfg
    outs = []
    for core in range(c.NCORES):
        o = results[core]["out"]                      # [nh, HCW, BC]
        outs.append(o.reshape(c.H, c.BC).T)           # [BC, H]
    return np.ascontiguousarray(np.concatenate(outs, axis=0))


_CACHE = {}


def run(inputs, trace=False, **kw):
    from concourse.bass_utils import run_bass_kernel_spmd

    cfg = Cfg()
    in_maps = host_prepare(cfg, **{k: inputs[k] for k in (
        "words", "subj_pos", "obj_pos", "emb_table", "w1", "b1", "w2", "b2",
        "mw1", "mb1", "mw2", "mb2")})
    if "nc" not in _CACHE:
        _CACHE["nc"] = build_nc(cfg)
    nc = _CACHE["nc"]
    res = run_bass_kernel_spmd(nc, in_maps, core_ids=list(range(cfg.NCORES)),
                               trace=trace, **kw)
    return assemble_output(cfg, res.results), res


def kernel(**inputs) -> np.ndarray:
    return run(inputs)[0]



# revision 4
# speedup vs baseline: 1.1664x; 1.1664x over previous
"""Trainium2 Bass kernel for nn_AttentionNet (pooling / ridge regime).

Model (per batch b of B=128, L=512, D=300, H=200, V=50000):
  word_emb = emb_table[words]                          [B,L,D]
  subj_emb = max over l with subj_pos[b,l]==0 of word_emb (else -1e12)
  obj_emb  = same with obj_pos
  hid  = tanh(word_emb @ w1[:D] + subj_emb @ w1[D:] + b1)
  attn = softmax(hid @ w2, axis=l)    (b2 dropped: softmax shift-invariant)
  subj_attn = sum_l attn * word_emb   (obj_attn identical -- source bug)
  out = relu(relu(cat([subj_attn, subj_attn, subj_emb, obj_emb]) @ mw1 + mb1) @ mw2 + mb2)

Sharding: pure data parallel, 16 batches per core on 8 cores; embedding
table and the small weights replicated.

All embeddings/weights are bf16 on device (PE at 1 cyc/row instead of
fp32's 4, half the gather bytes, 2x DVE); PSUM accumulation, softmax
stats, biases and the final output stay fp32.

Device plan per core (16 batches = 16 token-tiles of 512):
  - bulk gather via gpsimd.dma_gather (int16 indices).  The vocabulary
    exceeds int16 range, so each batch's tokens are sorted by word id
    (attention + pools are order-invariant within a batch) and split into
    the 256 smallest / 256 largest; the low halves of all batches are
    gathered from table[0:32768] and the high halves from
    table[V-32768:V], giving pure int16 indices with zero waste.
  - masked max-pools computed from the gathered embeddings: per-token
    additive masks (-2e12 for suppressed positions) via ACT/GPSIMD, a
    pairwise max tree on DVE/GPSIMD, PE transposes, a segmented
    reduce_max, then a final clamp to -1e12 which restores bit-exact
    semantics even for all-masked rows.
  - attention scores via bf16 matmuls on D-major PE-transposed
    embeddings, softmax on-chip, attention-weighted sum with the
    attention column as the stationary matmul operand.
  - 2-layer output MLP with the duplicated subj_attn block pre-folded
    into mw1 on the host (rows 0:300 += rows 300:600).
"""

import numpy as np

import concourse.bass as bass
import concourse.bacc as bacc
import concourse.mybir as mybir
import concourse.tile as tile
from concourse.masks import make_identity
from contextlib import ExitStack

F32 = mybir.dt.float32
BF16 = mybir.dt.bfloat16
I16 = mybir.dt.int16

NEG_INF = 1e12      # reference constant
MASK_ADD = -2e12    # additive mask; clamped back to -NEG_INF at the end

# ---------------------------------------------------------------- config


class Cfg:
    def __init__(self, B=128, L=512, D=300, H=200, V=50000, NCORES=8,
                 PT=128, CW=128, HCW=100, gather_split=4):
        self.B, self.L, self.D, self.H, self.V = B, L, D, H, V
        self.NCORES = NCORES
        self.BC = B // NCORES          # batches per core
        self.PT = PT                   # token subtile (partitions)
        self.NSUB = L // PT            # subtiles per batch (must be even)
        self.NS = self.BC * self.NSUB  # token subtiles per core
        self.T = self.BC * L           # tokens per core
        self.CW = CW                   # D-chunk width
        self.HCW = HCW                 # H-chunk width
        self.gather_split = gather_split
        assert L % PT == 0 and H % HCW == 0 and self.NSUB % 2 == 0
        # gather element size in bf16 elements: row bytes padded to 256B
        self.E = -(-D * 2 // 256) * 128          # 384 for D=300
        # int16-addressable split of the vocabulary
        self.LO_MAX = min(V, 32768)    # low table = rows [0, LO_MAX)
        self.HB = max(V - 32768, 0)    # high table = rows [HB, V)
        # exact chunks of D (last may be narrow)
        self.dch = []
        s = 0
        while s < D:
            self.dch.append((s, min(CW, D - s)))
            s += CW
        self.hch = [(i * HCW, HCW) for i in range(H // HCW)]
        self.nd = len(self.dch)
        self.nh = len(self.hch)
        assert self.nd * CW == self.E  # bf16 rows tile exactly into chunks

    def subtiles(self, b):
        """Global subtile ids of batch b: low half then high half."""
        h = self.NSUB // 2
        lo = [h * b + k for k in range(h)]
        hi = [self.NS // 2 + h * b + k for k in range(h)]
        return lo + hi


# ------------------------------------------------------------- device IR


def build_nc(cfg: Cfg):
    c = cfg
    nc = bacc.Bacc(num_swdge_queues=4)

    NH16 = (c.T // 2) // 16
    idxlo_d = nc.declare_dram_parameter("idx_lo", [128, NH16], I16, isOutput=False)
    idxhi_d = nc.declare_dram_parameter("idx_hi", [128, NH16], I16, isOutput=False)
    table = nc.declare_dram_parameter("table", [c.V, c.E], BF16, isOutput=False)
    madd_d = nc.declare_dram_parameter("madd", [c.PT, 2, c.BC, c.NSUB], F32, isOutput=False)
    w1a_d = nc.declare_dram_parameter("w1a", [c.D, c.H], BF16, isOutput=False)
    w1b_d = nc.declare_dram_parameter("w1b", [c.D, c.H], BF16, isOutput=False)
    b1_d = nc.declare_dram_parameter("b1", [c.H, 1], F32, isOutput=False)
    w2_d = nc.declare_dram_parameter("w2", [c.H, 1], BF16, isOutput=False)
    mw1_d = nc.declare_dram_parameter("mw1e", [3 * c.D, c.H], BF16, isOutput=False)
    mb1_d = nc.declare_dram_parameter("mb1", [c.H, 1], F32, isOutput=False)
    mw2_d = nc.declare_dram_parameter("mw2", [c.H, c.H], BF16, isOutput=False)
    mb2_d = nc.declare_dram_parameter("mb2", [c.H, 1], F32, isOutput=False)
    out_d = nc.declare_dram_parameter("out", [c.nh, c.HCW, c.BC], F32, isOutput=True)

    with tile.TileContext(nc) as tc, ExitStack() as ctx:
        sb = ctx.enter_context(tc.tile_pool(name="sb", bufs=1))

        # ---- persistent SBUF tiles
        ixl_sb = sb.tile([128, NH16], I16)
        ixh_sb = sb.tile([128, NH16], I16)
        emb_tok = sb.tile([c.PT, c.NS, c.E], BF16)
        madd_sb = sb.tile([c.PT, 2, c.BC, c.NSUB], F32)
        w1a_sb = sb.tile([c.CW, c.nd, c.H], BF16)
        w1b_sb = sb.tile([c.CW, c.nd, c.H], BF16)
        w2_sb = sb.tile([c.HCW, c.nh], BF16)
        b1_sb = sb.tile([c.HCW, c.nh], F32)
        mw1_sb = sb.tile([c.CW, 3 * c.nd, c.H], BF16)
        mb1_sb = sb.tile([c.HCW, c.nh], F32)
        mw2_sb = sb.tile([c.HCW, c.nh, c.H], BF16)
        mb2_sb = sb.tile([c.HCW, c.nh], F32)
        ident = sb.tile([c.PT, c.PT], BF16)
        pooled = sb.tile([c.CW, 2, c.BC, c.nd], BF16)   # [dlow, mask, b, chunk]
        bias_sb = sb.tile([c.HCW, c.nh, c.BC], F32)
        scores = sb.tile([c.BC, c.L], F32)
        attn = sb.tile([c.BC, c.L], F32)
        attn_bf = sb.tile([c.BC, c.L], BF16)
        attn_t = sb.tile([c.PT, c.NSUB, c.BC], BF16)
        sattn = sb.tile([c.BC, c.D], F32)
        sattn_bf = sb.tile([c.BC, c.D], BF16)
        satd = sb.tile([c.CW, c.nd, c.BC], BF16)
        smax = sb.tile([c.BC, 1], F32)
        nsmax = sb.tile([c.BC, 1], F32)
        ssum = sb.tile([c.BC, 1], F32)
        srec = sb.tile([c.BC, 1], F32)
        o1_sb = sb.tile([c.HCW, c.nh, c.BC], BF16)
        out_sb = sb.tile([c.HCW, c.nh, c.BC], F32)

        # ---- load indices & weights
        nc.sync.dma_start(out=ixl_sb[:], in_=idxlo_d[:])
        nc.sync.dma_start(out=ixh_sb[:], in_=idxhi_d[:])
        nc.sync.dma_start(out=madd_sb[:], in_=madd_d[:])
        for ci, (d0, dn) in enumerate(c.dch):
            nc.sync.dma_start(out=w1a_sb[0:dn, ci, :], in_=w1a_d[d0:d0 + dn, :])
            nc.sync.dma_start(out=w1b_sb[0:dn, ci, :], in_=w1b_d[d0:d0 + dn, :])
            for blk in range(3):
                nc.sync.dma_start(out=mw1_sb[0:dn, blk * c.nd + ci, :],
                                  in_=mw1_d[blk * c.D + d0:blk * c.D + d0 + dn, :])
        for hi, (h0, hn) in enumerate(c.hch):
            nc.sync.dma_start(out=w2_sb[0:hn, hi:hi + 1], in_=w2_d[h0:h0 + hn, :])
            nc.sync.dma_start(out=b1_sb[0:hn, hi:hi + 1], in_=b1_d[h0:h0 + hn, :])
            nc.sync.dma_start(out=mb1_sb[0:hn, hi:hi + 1], in_=mb1_d[h0:h0 + hn, :])
            nc.sync.dma_start(out=mb2_sb[0:hn, hi:hi + 1], in_=mb2_d[h0:h0 + hn, :])
            nc.sync.dma_start(out=mw2_sb[0:hn, hi, :], in_=mw2_d[h0:h0 + hn, :])
        make_identity(nc, ident[:])

        # ---- bulk gathers: low halves -> subtiles [0, NS/2), high halves after
        NHALF = c.T // 2
        nsp = c.gather_split
        npc = NHALF // nsp
        assert npc % 128 == 0, (NHALF, nsp)
        for k in range(nsp):
            i0, s0 = k * (npc // 16), k * (npc // 128)
            nc.gpsimd.dma_gather(
                out_ap=emb_tok[:, s0:s0 + npc // 128, :], in_ap=table[0:c.LO_MAX, :],
                idxs_ap=ixl_sb[:, i0:i0 + npc // 16], num_idxs=npc, num_idxs_reg=npc,
                elem_size=c.E, single_packet=False, queue_num=(2 * k) % 4)
            nc.gpsimd.dma_gather(
                out_ap=emb_tok[:, c.NS // 2 + s0:c.NS // 2 + s0 + npc // 128, :],
                in_ap=table[c.HB:c.V, :],
                idxs_ap=ixh_sb[:, i0:i0 + npc // 16], num_idxs=npc, num_idxs_reg=npc,
                elem_size=c.E, single_packet=False, queue_num=(2 * k + 1) % 4)

        # ---- grouped main loop: pools -> group bias -> dense hid/scores.
        # Groups of GB batches keep the PE stream dense (HAM stays warm) and
        # let the ACT/DVE pool work of group g+1 overlap the PE work of g.
        GB = min(4, c.BC)
        NG = c.BC // GB

        def pool_rhs(m, ci, bsl=slice(None)):
            dn = c.dch[ci][1]
            return pooled[0:dn, m, bsl, ci]

        with tc.tile_pool(name="mkpool", bufs=3) as mkpool, \
             tc.tile_pool(name="ppool", bufs=1, space="PSUM") as ppool, \
             tc.tile_pool(name="bpool", bufs=1, space="PSUM") as bpool, \
             tc.tile_pool(name="tpool", bufs=1, space="PSUM") as tpool, \
             tc.tile_pool(name="hpool", bufs=2, space="PSUM") as hpool, \
             tc.tile_pool(name="spool", bufs=1, space="PSUM") as spool, \
             tc.tile_pool(name="epool", bufs=2) as epool, \
             tc.tile_pool(name="hspool", bufs=2) as hspool, \
             tc.tile_pool(name="srpool", bufs=3) as srpool:
            for g in range(NG):
                gsl = slice(g * GB, (g + 1) * GB)
                # -- pools for the group
                for b in range(g * GB, (g + 1) * GB):
                    subs = c.subtiles(b)
                    for m in range(2):
                        masked = mkpool.tile([c.PT, c.NSUB, c.E], BF16, tag="masked")
                        for si, s in enumerate(subs):
                            if m == 0:
                                nc.scalar.activation(
                                    out=masked[:, si, :],
                                    in_=emb_tok[:, s, :],
                                    func=mybir.ActivationFunctionType.Identity,
                                    bias=madd_sb[:, m, b, si:si + 1], scale=1.0)
                            else:
                                nc.vector.tensor_scalar(
                                    out=masked[:, si, :],
                                    in0=emb_tok[:, s, :],
                                    scalar1=madd_sb[:, m, b, si:si + 1],
                                    scalar2=None, op0=mybir.AluOpType.add)
                        h = c.NSUB // 2
                        t1 = mkpool.tile([c.PT, h, c.E], BF16, tag="t1")
                        nc.vector.tensor_tensor(out=t1[:], in0=masked[:, 0:h, :],
                                                in1=masked[:, h:c.NSUB, :],
                                                op=mybir.AluOpType.max)
                        for q in range(h.bit_length() - 1):
                            hh = h >> (q + 1)
                            nc.vector.tensor_tensor(
                                out=t1[:, 0:hh, :], in0=t1[:, 0:hh, :],
                                in1=t1[:, hh:2 * hh, :], op=mybir.AluOpType.max)
                        pp = ppool.tile([c.CW, c.nd, c.PT], BF16, tag="pp")
                        for ci in range(c.nd):
                            nc.tensor.transpose(
                                out=pp[:, ci, :],
                                in_=t1[:, 0, ci * c.CW:(ci + 1) * c.CW],
                                identity=ident[:])
                        nc.vector.tensor_reduce(
                            out=pooled[:, m, b, :], in_=pp[:],
                            axis=mybir.AxisListType.X, op=mybir.AluOpType.max)
                # -- clamp restores exact -1e12 for all-masked rows
                nc.vector.tensor_scalar_max(
                    out=pooled[:, :, gsl, :], in0=pooled[:, :, gsl, :],
                    scalar1=-NEG_INF)
                # -- tanh bias for the group: w1b^T subj_emb + b1
                for hi, (h0, hn) in enumerate(c.hch):
                    pb = bpool.tile([c.HCW, GB], F32, tag="pb")
                    for ci, (d0, dn) in enumerate(c.dch):
                        nc.tensor.matmul(
                            out=pb[0:hn, :],
                            lhsT=w1b_sb[0:dn, ci, h0:h0 + hn],
                            rhs=pool_rhs(0, ci, gsl),
                            start=(ci == 0), stop=(ci == c.nd - 1))
                    nc.scalar.activation(
                        out=bias_sb[0:hn, hi, gsl], in_=pb[0:hn, :],
                        func=mybir.ActivationFunctionType.Identity,
                        bias=b1_sb[0:hn, hi:hi + 1], scale=1.0)
                # -- D-major transposes + copies for the group
                embds = []
                for b in range(g * GB, (g + 1) * GB):
                    subs = c.subtiles(b)
                    pt = tpool.tile([c.CW, c.nd, c.L], BF16, tag="pt")
                    for ci in range(c.nd):
                        for si, s in enumerate(subs):
                            nc.tensor.transpose(
                                out=pt[:, ci, si * c.PT:(si + 1) * c.PT],
                                in_=emb_tok[:, s, ci * c.CW:(ci + 1) * c.CW],
                                identity=ident[:])
                    emb_d = epool.tile([c.CW, c.nd, c.L], BF16, tag="embd")
                    nc.scalar.copy(out=emb_d[:, 0, :], in_=pt[:, 0, :])
                    nc.vector.tensor_copy(out=emb_d[:, 1:c.nd, :], in_=pt[:, 1:c.nd, :])
                    embds.append(emb_d)
                # -- dense hid + scores matmul stream for the group
                for bi, b in enumerate(range(g * GB, (g + 1) * GB)):
                    emb_d = embds[bi]
                    hid = hspool.tile([c.HCW, c.nh, c.L], BF16, tag="hid")
                    for hi, (h0, hn) in enumerate(c.hch):
                        ph = hpool.tile([c.HCW, c.L], F32, tag="ph")
                        for ci, (d0, dn) in enumerate(c.dch):
                            nc.tensor.matmul(
                                out=ph[0:hn, :],
                                lhsT=w1a_sb[0:dn, ci, h0:h0 + hn],
                                rhs=emb_d[0:dn, ci, :],
                                start=(ci == 0), stop=(ci == c.nd - 1))
                        nc.scalar.activation(
                            out=hid[0:hn, hi, :], in_=ph[0:hn, :],
                            func=mybir.ActivationFunctionType.Tanh,
                            bias=bias_sb[0:hn, hi, b:b + 1], scale=1.0)
                    ps = spool.tile([1, c.L], F32, tag="ps")
                    for hi, (h0, hn) in enumerate(c.hch):
                        nc.tensor.matmul(
                            out=ps[:], lhsT=w2_sb[0:hn, hi:hi + 1],
                            rhs=hid[0:hn, hi, :],
                            start=(hi == 0), stop=(hi == c.nh - 1))
                    srow = srpool.tile([1, c.L], F32, tag="srow")
                    nc.vector.tensor_copy(out=srow[:], in_=ps[:])
                    nc.sync.dma_start(out=scores[b:b + 1, :], in_=srow[:])

        # ---- softmax over L for all batches
        nc.vector.tensor_reduce(out=smax[:], in_=scores[:],
                                axis=mybir.AxisListType.X, op=mybir.AluOpType.max)
        nc.vector.tensor_scalar_mul(out=nsmax[:], in0=smax[:], scalar1=-1.0)
        nc.scalar.activation(out=attn[:], in_=scores[:],
                             func=mybir.ActivationFunctionType.Exp,
                             bias=nsmax[:, 0:1], scale=1.0)
        nc.vector.tensor_reduce(out=ssum[:], in_=attn[:],
                                axis=mybir.AxisListType.X, op=mybir.AluOpType.add)
        nc.vector.reciprocal(out=srec[:], in_=ssum[:])
        nc.vector.tensor_scalar_mul(out=attn[:], in0=attn[:], scalar1=srec[:, 0:1])
        nc.vector.tensor_copy(out=attn_bf[:], in_=attn[:])

        # ---- transpose attn to token-major columns [PT, si, b]
        # column layout: attn[b, si*PT:...] -> attn_t[:, si, b]
        with tc.tile_pool(name="apool", bufs=2, space="PSUM") as apool:
            for si in range(c.NSUB):
                pa = apool.tile([c.PT, c.BC], BF16, tag="pa")
                nc.tensor.transpose(out=pa[:],
                                    in_=attn_bf[:, si * c.PT:(si + 1) * c.PT],
                                    identity=ident[0:c.BC, 0:c.BC])
                nc.vector.tensor_copy(out=attn_t[:, si, :], in_=pa[:])

        # ---- attention-weighted sum  -> sattn [b, D]
        with tc.tile_pool(name="wpool", bufs=4, space="PSUM") as wpool, \
             tc.tile_pool(name="wrpool", bufs=3) as wrpool:
            for b in range(c.BC):
                subs = c.subtiles(b)
                pw = wpool.tile([1, c.D], F32, tag="pw")
                for si, s in enumerate(subs):
                    nc.tensor.matmul(
                        out=pw[:],
                        lhsT=attn_t[:, si, b:b + 1],
                        rhs=emb_tok[:, s, 0:c.D],
                        start=(si == 0), stop=(si == c.NSUB - 1))
                wrow = wrpool.tile([1, c.D], F32, tag="wrow")
                nc.scalar.copy(out=wrow[:], in_=pw[:])
                nc.sync.dma_start(out=sattn[b:b + 1, :], in_=wrow[:])

        # ---- transpose sattn to D-major chunks [dlow, chunk, b]
        nc.vector.tensor_copy(out=sattn_bf[:], in_=sattn[:])
        with tc.tile_pool(name="stpool", bufs=2, space="PSUM") as stpool:
            for ci, (d0, dn) in enumerate(c.dch):
                pst = stpool.tile([c.CW, c.BC], BF16, tag="pst")
                nc.tensor.transpose(out=pst[0:dn, :], in_=sattn_bf[:, d0:d0 + dn],
                                    identity=ident[0:c.BC, 0:c.BC])
                nc.vector.tensor_copy(out=satd[0:dn, ci, :], in_=pst[0:dn, :])

        # ---- output MLP (N=BC is small)
        with tc.tile_pool(name="mpool", bufs=2, space="PSUM") as mpool, \
             tc.tile_pool(name="m2pool", bufs=2, space="PSUM") as m2pool:
            nk = 3 * c.nd
            for hi, (h0, hn) in enumerate(c.hch):
                pm = mpool.tile([c.HCW, c.BC], F32, tag="pm")
                for blk in range(3):
                    for ci, (d0, dn) in enumerate(c.dch):
                        k = blk * c.nd + ci
                        if blk == 0:
                            rhs = satd[0:dn, ci, :]
                        else:
                            rhs = pool_rhs(blk - 1, ci)
                        nc.tensor.matmul(
                            out=pm[0:hn, :],
                            lhsT=mw1_sb[0:dn, k, h0:h0 + hn],
                            rhs=rhs, start=(k == 0), stop=(k == nk - 1))
                nc.scalar.activation(
                    out=o1_sb[0:hn, hi, :], in_=pm[0:hn, :],
                    func=mybir.ActivationFunctionType.Relu,
                    bias=mb1_sb[0:hn, hi:hi + 1], scale=1.0)
            for hi, (h0, hn) in enumerate(c.hch):
                pm2 = m2pool.tile([c.HCW, c.BC], F32, tag="pm2")
                for ki, (k0, kn) in enumerate(c.hch):
                    nc.tensor.matmul(
                        out=pm2[0:hn, :],
                        lhsT=mw2_sb[0:kn, ki, h0:h0 + hn],
                        rhs=o1_sb[0:kn, ki, :],
                        start=(ki == 0), stop=(ki == c.nh - 1))
                nc.scalar.activation(
                    out=out_sb[0:hn, hi, :], in_=pm2[0:hn, :],
                    func=mybir.ActivationFunctionType.Relu,
                    bias=mb2_sb[0:hn, hi:hi + 1], scale=1.0)
            for hi in range(c.nh):
                nc.sync.dma_start(out=out_d[hi], in_=out_sb[:, hi, :])

    nc.finalize()
    return nc


# ------------------------------------------------------------------ host


def wrap16(idx, n):
    """int16 index list -> [128, n/16] wrapped + replicated per Q7 core."""
    return np.ascontiguousarray(
        np.tile(idx.astype(np.int16).reshape(n // 16, 16).T, (8, 1)))


def to_bf16(x):
    import ml_dtypes
    return np.asarray(x, dtype=np.float32).astype(ml_dtypes.bfloat16)


def host_prepare(cfg: Cfg, words, subj_pos, obj_pos, emb_table,
                 w1, b1, w2, b2, mw1, mb1, mw2, mb2):
    import ml_dtypes
    c = cfg
    words = np.asarray(words).astype(np.int64)
    subj_pos = np.asarray(subj_pos)
    obj_pos = np.asarray(obj_pos)
    f32 = lambda x: np.ascontiguousarray(np.asarray(x, dtype=np.float32))

    table = np.zeros((c.V, c.E), ml_dtypes.bfloat16)
    table[:, :c.D] = to_bf16(emb_table)

    w1 = np.asarray(w1, dtype=np.float32)
    w1a, w1b = w1[:c.D], w1[c.D:2 * c.D]
    mw1 = np.asarray(mw1, dtype=np.float32)
    mw1e = np.concatenate([mw1[0:c.D] + mw1[c.D:2 * c.D],
                           mw1[2 * c.D:3 * c.D], mw1[3 * c.D:4 * c.D]], axis=0)
    shared = {
        "table": table,
        "w1a": to_bf16(w1a), "w1b": to_bf16(w1b),
        "b1": f32(b1).reshape(c.H, 1),
        "w2": to_bf16(np.asarray(w2).reshape(c.H, 1)),
        "mw1e": to_bf16(mw1e),
        "mb1": f32(mb1).reshape(c.H, 1),
        "mw2": to_bf16(mw2),
        "mb2": f32(mb2).reshape(c.H, 1),
    }
    HALF = c.L // 2
    in_maps = []
    for core in range(c.NCORES):
        b0 = core * c.BC
        lo_list, hi_list = [], []
        madd = np.zeros((c.PT, 2, c.BC, c.NSUB), np.float32)
        for b in range(c.BC):
            w = words[b0 + b]
            order = np.argsort(w, kind="stable")
            ws = w[order]
            if ws[HALF - 1] >= c.LO_MAX or ws[HALF] < c.HB:
                raise RuntimeError(
                    f"batch {b0 + b}: vocab split infeasible "
                    f"(lo_max={ws[HALF - 1]}, hi_min={ws[HALF]})")
            lo_list.append(ws[:HALF])
            hi_list.append(ws[HALF:] - c.HB)
            # mask addends follow the same permutation; token rank r sits at
            # subtile si=r//PT (low half) / NSUB/2 + (r-HALF)//PT, partition r%PT
            for m, pos in ((0, subj_pos), (1, obj_pos)):
                pm = (np.asarray(pos[b0 + b])[order] != 0)
                av = np.where(pm, np.float32(MASK_ADD), np.float32(0.0))
                madd[:, m, b, :] = av.reshape(c.NSUB, c.PT).T
        idx_lo = np.concatenate(lo_list)
        idx_hi = np.concatenate(hi_list)
        in_maps.append({
            "idx_lo": wrap16(idx_lo, c.T // 2),
            "idx_hi": wrap16(idx_hi, c.T // 2),
            "madd": np.ascontiguousarray(madd),
            **shared})
    return in_maps


def assemble_output(cfg: Cfg, results):
    c = cfg
    outs = []
    for core in range(c.NCORES):
        o = results[core]["out"]                      # [nh, HCW, BC]
        outs.append(o.reshape(c.H, c.BC).T)           # [BC, H]
    return np.ascontiguousarray(np.concatenate(outs, axis=0))


_CACHE = {}


def run(inputs, trace=False, **kw):
    from concourse.bass_utils import run_bass_kernel_spmd

    cfg = Cfg()
    in_maps = host_prepare(cfg, **{k: inputs[k] for k in (
        "words", "subj_pos", "obj_pos", "emb_table", "w1", "b1", "w2", "b2",
        "mw1", "mb1", "mw2", "mb2")})
    if "nc" not in _CACHE:
        _CACHE["nc"] = build_nc(cfg)
    nc = _CACHE["nc"]
    res = run_bass_kernel_spmd(nc, in_maps, core_ids=list(range(cfg.NCORES)),
                               trace=trace, **kw)
    return assemble_output(cfg, res.results), res


def kernel(**inputs) -> np.ndarray:
    return run(inputs)[0]
